# revision 1
# baseline (speedup 1.0000x reference)
"""Trainium2 Bass kernel for nn_BatchEKFR (leaky-integrator network with
nonlinear firing-rate feedback).

Math: the per-step state v[b,m,h] is linear in its drivers, so
  z_t[b,m] = sum_s AW_{t-s}[m] cur_s[b,m] + sum_s BW_{t-1-s}[m] fs_s[b,m]
with AW_k[m] = sum_h decay^k a w, BW_k[m] = sum_h decay^k (1000 b) w.
The currents part is a per-m causal Toeplitz matmul (parallel, on the PE);
only the activation->feedback chain is sequential (DVE/ACT), applied as a
rank-1 update into all future time slots after each step.

Sharding: model dim M=512 split across 8 cores (64 models each) — fully
independent, no collectives. Each core's (b, m) pairs are laid out as
128 partitions = (m_local, b_hi) x 16 lanes (b_lo).
"""
import numpy as np

T, B, M, H = 128, 32, 512, 64
NCORES = 8
ML = M // NCORES   # 64 local models
BL, BH = 16, 2     # b = bh*16 + bl ; partition p = m*2 + bh
WID = BL * 128     # 2048
NW = T - 1         # inject offsets


def _host_precompute(inputs):
    import concourse.mybir as mybir  # noqa: F401  (ensures env importable early)
    cur = np.asarray(inputs['currents'], np.float32)
    a = np.asarray(inputs['a'], np.float32)
    b = np.asarray(inputs['b'], np.float32)
    w = np.asarray(inputs['w'], np.float32)
    ds = np.asarray(inputs['ds'], np.float32)
    g_b = np.asarray(inputs['g_b'], np.float32)
    mc = np.asarray(inputs['max_current'], np.float32)
    mfr = np.asarray(inputs['max_firing_rate'], np.float32)
    pc = np.asarray(inputs['poly_coeff'], np.float32)

    decay = (1.0 - ds).astype(np.float32)
    Dp = (decay[None, :].astype(np.float64) ** np.arange(T)[:, None]).astype(np.float32)
    inv_mc = (1.0 / mc).astype(np.float32)
    AWf = (np.einsum('kh,mh->km', Dp, (a * w)) * inv_mc[None, :]).astype(np.float32)
    BWf = (np.einsum('kh,mh->km', Dp, (1000.0 * b * w)) * (inv_mc * mfr)[None, :]).astype(np.float32)
    bias0 = (-g_b * inv_mc).astype(np.float32)
    c = (pc ** 2).astype(np.float32)

    in_maps = []
    for cix in range(NCORES):
        sl = slice(cix * ML, (cix + 1) * ML)
        awf, bwf = AWf[:, sl], BWf[:, sl]
        cs = cur[:, :, sl].reshape(T, BH, BL, ML)
        curT = np.ascontiguousarray(
            cs.transpose(0, 2, 3, 1).reshape(T, BL * ML * BH)).astype(np.float32)
        ks = np.arange(T)[None, :] - np.arange(T)[:, None]
        toep = np.where((ks >= 0)[None, :, :], awf.T[:, np.clip(ks, 0, T - 1)], 0.0)
        awt = np.ascontiguousarray(toep.transpose(1, 0, 2).reshape(T, ML * T)).astype(np.float32)
        bw_p = np.repeat(bwf.T, BH, axis=0)
        bwrep = np.ascontiguousarray(
            np.repeat(bw_p[:, :NW, None], BL, axis=2).reshape(128, NW * BL)).astype(np.float32)
        cp = c[sl]
        pp = np.zeros((128, 8), np.float32)
        pp[:, 0] = np.repeat(cp[:, 3], BH); pp[:, 1] = np.repeat(cp[:, 2], BH)
        pp[:, 2] = np.repeat(cp[:, 1], BH); pp[:, 3] = np.repeat(cp[:, 0], BH)
        pp[:, 4] = np.repeat(bias0[sl], BH)
        pp[:, 5] = np.repeat(mfr[sl].astype(np.float32), BH)
        ident = np.ascontiguousarray(np.tile(np.eye(32, dtype=np.float32), (4, 1)))
        inbuf = np.concatenate([curT, awt, bwrep, pp, ident], axis=1)
        in_maps.append(dict(inbuf=inbuf))
    return in_maps


_NC_CACHE = None


def _build_program():
    global _NC_CACHE
    if _NC_CACHE is not None:
        return _NC_CACHE
    import concourse.bacc as bacc
    import concourse.mybir as mybir
    from concourse.tile import TileContext

    F32 = mybir.dt.float32
    AF = mybir.ActivationFunctionType
    OP = mybir.AluOpType

    nc = bacc.Bacc()
    NIN = WID + ML * T + NW * BL + 8 + 32
    in_d = nc.declare_dram_parameter("inbuf", [128, NIN], F32, isOutput=False)
    out_d = nc.declare_dram_parameter("fs_raw", [128, WID], F32, isOutput=True)

    with TileContext(nc) as tc:
        with (
            tc.tile_pool(name="const", bufs=1) as cpool,
            tc.tile_pool(name="work", bufs=1) as wpool,
            tc.tile_pool(name="psum", bufs=1, space="PSUM") as ppool,
        ):
            inb = cpool.tile([128, NIN], F32, tag="inbuf")
            nc.sync.dma_start(inb[:], in_d[:])
            o = 0
            curT = inb[:, o:o + WID]; o += WID
            awt = inb[:, o:o + ML * T]; o += ML * T
            bwrep = inb[:, o:o + NW * BL]; o += NW * BL
            pp = inb[:, o:o + 8]; o += 8
            ident = inb[:, o:o + 32]; o += 32

            # currents Toeplitz conv: yT col = m*32 + bl*2 + bh
            yT = ppool.tile([T, WID], F32, tag="yT")
            curT3 = curT.rearrange("t (bl q) -> t bl q", bl=BL)
            yT3 = yT[:].rearrange("t (m c) -> t m c", m=ML)
            for m in range(ML):
                nc.tensor.matmul(
                    yT3[:, m, :],
                    awt[:, m * T:(m + 1) * T],
                    curT3[:, :, 2 * m:2 * m + 2],
                    start=True, stop=True)

            # reorder into transpose-friendly layout: yS col = bl*128 + m*2 + bh
            yS = wpool.tile([T, WID], F32, tag="yS")
            yT_v = yT[:].rearrange("t (m bl q) -> t bl m q", m=ML, bl=BL)
            yS_v = yS[:].rearrange("t (bl m q) -> t bl m q", bl=BL, m=ML)
            nc.scalar.copy(yS_v, yT_v)

            # transpose to chain layout: zsbuf [128, 2048] col = 16*t + bl
            zsbuf = wpool.tile([128, 16 * T], F32, tag="zsbuf")
            zseq = ppool.tile([128, 512], F32, tag="zseq")
            for j2 in range(4):
                for bl in range(BL):
                    tp = (96, 0) if 32 * j2 == 96 else None
                    nc.tensor.transpose(
                        zseq[:, bl * 32:(bl + 1) * 32],
                        yS[32 * j2:32 * j2 + 32, bl * 128:(bl + 1) * 128],
                        ident[32 * j2:32 * j2 + 32, :32], tile_position=tp)
                zq_in = zseq[:].rearrange("p (bl k) -> p k bl", bl=BL)
                zs_out = zsbuf[:, 512 * j2:512 * (j2 + 1)].rearrange(
                    "p (k bl) -> p k bl", bl=BL)
                nc.vector.tensor_scalar(zs_out, zq_in, pp[:, 4:5], None, OP.add)

            # sequential activation/feedback chain
            fsH = wpool.tile([128, BL * T], F32, tag="fsH")   # col = bl*128 + t
            tu = wpool.tile([128, BL], F32, tag="tu")
            tv = wpool.tile([128, BL], F32, tag="tv")
            tsq = wpool.tile([128, BL], F32, tag="tsq")
            tp1 = wpool.tile([128, BL], F32, tag="tp1")
            tp2 = wpool.tile([128, BL], F32, tag="tp2")
            tth = wpool.tile([128, BL], F32, tag="tth")
            tmpi = wpool.tile([128, NW * BL], F32, tag="tmpi")
            fsH3 = fsH[:].rearrange("p (bl s) -> p bl s", bl=BL)
            bwr3 = bwrep.rearrange("p (j bl) -> p j bl", bl=BL)
            tmpi3 = tmpi[:].rearrange("p (j bl) -> p j bl", bl=BL)

            for t in range(T):
                x = zsbuf[:, BL * t:BL * (t + 1)]
                nc.vector.tensor_scalar(tu[:], x, pp[:, 0:1], pp[:, 1:2], OP.mult, OP.add)
                nc.vector.tensor_scalar(tv[:], x, pp[:, 2:3], pp[:, 3:4], OP.mult, OP.add)
                nc.vector.tensor_tensor(tsq[:], x, x, OP.mult)
                nc.vector.tensor_tensor(tp1[:], tu[:], tsq[:], OP.mult)
                nc.vector.tensor_tensor(tp2[:], tp1[:], tv[:], OP.add)
                nc.scalar.activation(tth[:], tp2[:], AF.Tanh)
                nc.vector.tensor_scalar(fsH3[:, :, t], tth[:], 0.0, None, OP.max)
                n = T - 1 - t
                if n > 0:
                    src = fsH3[:, :, t].unsqueeze(1).broadcast_to([128, n, BL])
                    nc.vector.tensor_tensor(tmpi3[:, :n, :], src, bwr3[:, :n, :], OP.mult)
                    nc.vector.tensor_tensor(
                        zsbuf[:, BL * (t + 1):BL * (t + 1 + n)],
                        zsbuf[:, BL * (t + 1):BL * (t + 1 + n)],
                        tmpi[:, :n * BL], OP.add)

            fso = wpool.tile([128, BL * T], F32, tag="fso")
            nc.vector.tensor_scalar(fso[:], fsH[:], pp[:, 5:6], None, OP.mult)
            nc.sync.dma_start(out_d[:], fso[:])
    nc.finalize()
    _NC_CACHE = nc
    return nc


def _fsraw_to_fs(fs_raw):
    r = np.asarray(fs_raw).reshape(ML, BH, BL, T)
    return np.ascontiguousarray(r.transpose(3, 1, 2, 0).reshape(T, B, ML))


def kernel(**inputs):
    from concourse.bass_utils import run_bass_kernel_spmd
    in_maps = _host_precompute(inputs)
    nc = _build_program()
    res = run_bass_kernel_spmd(nc, in_maps, list(range(NCORES)))
    out = np.concatenate(
        [_fsraw_to_fs(res.results[i]["fs_raw"]) for i in range(NCORES)], axis=2)
    return out.astype(np.float32)
